# revision 7
# baseline (speedup 1.0000x reference)
"""Trainium2 Bass kernel v4 for windowed multi-head attention (2.5D swin).

Problem (hardcoded from spec nn_Attention25d_86775519248925):
  x:          (4, 16, 16, 8, 7, 7, 1, 128) f32  -> B=8192 windows, n=49 tokens, d=128
  w_qkv:      (128, 384) f32
  w_out:      (128, 128) f32
  bias_table: (169, 4) f32
  out:        same shape as x

Sharding: pure data parallel over the fused window-batch axis across 8 cores.

v4 design (per core: 128 groups of 8 windows = 4 window-pairs,
token slots fully COMPACT: 98 = 2x49 per pair, no padding):
  - x is reshaped + bf16-cast on the HOST into xT [d=128, group, pair, 98]
    (a pure reshape/transpose -- tokens are contiguous), so the kernel
    needs no transposes, no cast-DMAs, no padding, and input HBM traffic
    is 0.38x of the f32 token-major original.
  - q,k: two shared-weight matmuls (N=392) into ONE 2-bank psum tile;
    qT copy on scalar, kT copy on vector (concurrent).
  - sim^T per (head, pair): one matmul, lhsT = kT 32-row head slice
    (tile_position=(32h,0)), 98-j output.  Two heads share one 2-bank
    psum tile (per-head banks satisfy the row-group/bank rule).
  - softmax per HEAD (fine-grained pipeline): exp on scalar (psum->sbuf
    bf16), multiply by host-precomputed exp(bias) on GPSIMD (sbuf-only
    engine; masking of cross-window entries is exact multiply-by-zero;
    |sim| <= ~0.4 so exp never overflows), then immediately the Z matmul
    (lhsT=ones[98,32], col-masked to partitions 32h..32h+32 -- broadcast
    over dh for free) and the 4 attn@v matmuls (K=98, col-masked).
  - reciprocal_approx_fast for 1/Z; applied in the psum->sbuf y copy as a
    single vector multiply.
  - final projection of group g is deferred until after group g+1's sim
    matmuls; outb copy is split scalar/vector halves.

PSUM banks (8): sim 2x2, qk 1x2, {v, fin, rz, y} ring 2x1.

Hardware constraints honored (probed in earlier versions):
  - concurrent tile-position matmuls from different row-groups must write
    different PSUM banks (per-head sim banks).
  - no PSUM accumulation chains across row-groups (attn@v contracts K=98
    in one matmul; cross-window attn entries are exactly 0).
  - GPSIMD cannot access PSUM (it only gets the sbuf-only eb multiply).
"""

import os
import sys
import threading

import numpy as np

for _p in ("/opt/trn_rl_repo", "/root/.axon_site/_ro/trn_rl_repo"):
    if os.path.isdir(_p) and _p not in sys.path:
        sys.path.insert(0, _p)

# ---------------------------------------------------------------- constants
WS = 7
N_TOK = 49            # tokens per window
D = 128
H = 4
DH = 32
SCALE = DH ** -0.5
B_FULL = 4 * 16 * 16 * 8   # 8192 windows
N_CORES = 8
NI = 2 * N_TOK        # 98 compact token slots per pair


def _rel_pos_bias(bias_table: np.ndarray) -> np.ndarray:
    """bias[h, i, j] from the 169x4 table (numpy copy of reference logic)."""
    pos = np.arange(WS)
    gi, gj = np.meshgrid(pos, pos, indexing="ij")
    grid = np.stack([gi.reshape(-1), gj.reshape(-1)], axis=-1)
    rel = grid[:, None, :] - grid[None, :, :] + (WS - 1)
    idx = rel[..., 0] * (2 * WS - 1) + rel[..., 1]            # (49, 49)
    b = bias_table[idx]                                       # (49, 49, 4)
    return np.transpose(b, (2, 0, 1)).astype(np.float32)      # (h, i, j)


def _host_constants(w_qkv, w_out, bias_table):
    import ml_dtypes
    bf = ml_dtypes.bfloat16
    wq = np.ascontiguousarray((w_qkv[:, :D] * SCALE).astype(bf))
    wk = np.ascontiguousarray(w_qkv[:, D:2 * D].astype(bf))
    wv = np.ascontiguousarray(w_qkv[:, 2 * D:].astype(bf))
    wo = np.ascontiguousarray(w_out.astype(bf))

    bias = _rel_pos_bias(np.asarray(bias_table, dtype=np.float32))  # (h,i,j)
    # eb4[j, h, p, i] = exp(bias) on the window-diagonal, 0 elsewhere
    # (cross-window masking by multiply).  j, i in [0, 98), 49 per window.
    b4 = np.zeros((NI, H, 4, NI), dtype=np.float32)
    ebT = np.exp(np.transpose(bias, (0, 2, 1)))               # (h, j_tok, i_tok)
    for w in range(2):
        b4[N_TOK * w: N_TOK * (w + 1), :, :, N_TOK * w: N_TOK * (w + 1)] = \
            ebT.transpose(1, 0, 2)[:, :, None, :]
    eb4 = b4.astype(bf)
    ones32 = np.ones((NI, DH), dtype=bf)
    return dict(wq=wq, wk=wk, wv=wv, wo=wo, eb4=eb4, ones32=ones32)


def _host_xT(x_tokens: np.ndarray, n_windows: int) -> np.ndarray:
    """xT [128(d), n_groups, 4(pair), 98(tok)] bf16 from token-major
    x [nt, 128] f32 for one core -- a pure reshape/transpose/cast."""
    import ml_dtypes
    bf = ml_dtypes.bfloat16
    n_groups = n_windows // 8
    xT = np.ascontiguousarray(
        x_tokens.reshape(n_groups, 4, NI, D).transpose(3, 0, 1, 2).astype(bf))
    return xT


def _build_bass(n_windows: int):
    """Build the Bass/Tile program for one core processing n_windows windows."""
    import concourse.bacc as bacc
    import concourse.bass as bass
    import concourse.mybir as mybir
    import concourse.tile as tile

    f32 = mybir.dt.float32
    bf = mybir.dt.bfloat16
    NT = n_windows * N_TOK
    n_groups = n_windows // 8
    assert n_windows % 8 == 0

    nc = bacc.Bacc("TRN2", target_bir_lowering=False, debug=False,
                   enable_asserts=False)

    xT_d = nc.dram_tensor("xT", [D, n_groups, 4, NI], bf, kind="ExternalInput")
    out_t = nc.dram_tensor("out", [NT, D], f32, kind="ExternalOutput")
    wq_d = nc.dram_tensor("wq", [D, D], bf, kind="ExternalInput")
    wk_d = nc.dram_tensor("wk", [D, D], bf, kind="ExternalInput")
    wv_d = nc.dram_tensor("wv", [D, D], bf, kind="ExternalInput")
    wo_d = nc.dram_tensor("wo", [D, D], bf, kind="ExternalInput")
    eb_d = nc.dram_tensor("eb4", [NI, H, 4, NI], bf, kind="ExternalInput")
    ones_d = nc.dram_tensor("ones32", [NI, DH], bf, kind="ExternalInput")

    with tile.TileContext(nc) as tc:
        with (
            tc.tile_pool(name="singles", bufs=1) as singles,
            tc.tile_pool(name="xt", bufs=4) as pool_xt,
            tc.tile_pool(name="qk", bufs=3) as pool_qk,
            tc.tile_pool(name="vsb", bufs=3) as pool_v,
            tc.tile_pool(name="attn", bufs=8) as pool_attn,
            tc.tile_pool(name="rz", bufs=2) as pool_rz,
            tc.tile_pool(name="ysb", bufs=3) as pool_y,
            tc.tile_pool(name="outb", bufs=3) as pool_out,
            tc.tile_pool(name="psS", bufs=2, space="PSUM") as pool_sim,
            tc.tile_pool(name="psQK", bufs=1, space="PSUM") as pool_pqk,
            tc.tile_pool(name="psV", bufs=2, space="PSUM") as pool_vfy,
        ):
            wq_sb = singles.tile([D, D], bf, tag="wq")
            wk_sb = singles.tile([D, D], bf, tag="wk")
            wv_sb = singles.tile([D, D], bf, tag="wv")
            wo_sb = singles.tile([D, D], bf, tag="wo")
            eb_sb = singles.tile([NI, H, 4, NI], bf, tag="eb")
            ones_sb = singles.tile([NI, DH], bf, tag="ones")
            for sb, dr in ((wq_sb, wq_d), (wk_sb, wk_d), (wv_sb, wv_d),
                           (wo_sb, wo_d), (eb_sb, eb_d), (ones_sb, ones_d)):
                nc.sync.dma_start(out=sb[:], in_=dr[:])

            def emit_fin(y_sb, g):
                ps_f = pool_vfy.tile([128, 4, 128], f32, tag="v")
                for p in range(4):
                    nc.tensor.matmul(ps_f[:NI, p, :], y_sb[:, p, :], wo_sb[:])
                outb = pool_out.tile([NI, 4, D], f32, tag="outb")
                nc.scalar.copy(outb[:], ps_f[:NI, :, :])
                tok0 = g * 392
                for p_ in range(2):
                    od_ap = bass.AP(
                        tensor=out_t, offset=(tok0 + p_ * N_TOK) * D,
                        ap=[[D, N_TOK], [2 * N_TOK * D, 4], [1, D]])
                    nc.sync.dma_start(
                        out=od_ap, in_=outb[N_TOK * p_: N_TOK * (p_ + 1)])

            pending = None
            for g in range(n_groups):
                # ---- input: straight DMA of host-pretransposed x ---------
                xT = pool_xt.tile([128, 4, NI], bf, tag="xt")
                nc.sync.dma_start(out=xT[:], in_=xT_d[:, g, :, :])

                # ---- q, k into one 2-bank tile; concurrent copies --------
                ps_qk = pool_pqk.tile([128, 2, 4, 128], f32, tag="qk")
                nc.tensor.matmul(ps_qk[:, 0, :, :NI], wq_sb[:], xT[:])
                nc.tensor.matmul(ps_qk[:, 1, :, :NI], wk_sb[:], xT[:])
                qT = pool_qk.tile([128, 4, NI], bf, tag="qT")
                nc.scalar.copy(qT[:], ps_qk[:, 0, :, :NI])
                kT = pool_qk.tile([128, 4, NI], bf, tag="kT")
                nc.vector.tensor_copy(kT[:], ps_qk[:, 1, :, :NI])

                # ---- sim^T: heads interleaved for row-group concurrency --
                sim_tiles = []
                for hh in range(2):           # head pair (2hh, 2hh+1)
                    ps_sim = pool_sim.tile([128, 2, 4, 128], f32, tag="s")
                    for p in range(4):
                        for hi in range(2):
                            h = 2 * hh + hi
                            nc.tensor.matmul(
                                ps_sim[:NI, hi, p, :NI],
                                kT[DH * h: DH * (h + 1), p, :],
                                qT[DH * h: DH * (h + 1), p, :],
                                tile_position=(DH * h, 0), start=True, stop=True)
                    sim_tiles.append(ps_sim)

                # ---- v token-major (pair-stationary xT) ------------------
                ps_v = pool_vfy.tile([128, 4, 128], f32, tag="v")
                for p in range(4):
                    nc.tensor.matmul(ps_v[:NI, p, :], xT[:, p, :], wv_sb[:])
                v_sb = pool_v.tile([NI, 4, 128], bf, tag="v")
                nc.vector.tensor_copy(v_sb[:], ps_v[:NI, :, :])

                # ---- deferred final projection of the previous group -----
                if pending is not None:
                    emit_fin(*pending)
                    pending = None

                # ---- per-head softmax chain + Z + attn@v -----------------
                ps_rz = pool_vfy.tile([128, 4, 128], f32, tag="v")
                ps_y = pool_vfy.tile([128, 4, 128], f32, tag="v")
                for h in range(H):
                    hh, hi = divmod(h, 2)
                    ps_sim = sim_tiles[hh]
                    attn = pool_attn.tile([NI, 4, NI], bf, tag="attn")
                    nc.scalar.activation(attn[:], ps_sim[:NI, hi, :, :NI],
                                         mybir.ActivationFunctionType.Exp)
                    eng = nc.vector if h == 3 else nc.gpsimd
                    eng.tensor_mul(attn[:], attn[:], eb_sb[:, h, :, :])
                    nc.tensor.matmul(
                        ps_rz[DH * h: DH * (h + 1), :, :NI],
                        ones_sb[:], attn[:],
                        tile_position=(0, DH * h), start=True, stop=True)
                    for p in range(4):
                        nc.tensor.matmul(
                            ps_y[DH * h: DH * (h + 1), p, :NI],
                            v_sb[:, p, DH * h: DH * (h + 1)],
                            attn[:, p, :],
                            tile_position=(0, DH * h), start=True, stop=True)

                rz2 = pool_rz.tile([128, 4, NI], f32, tag="rz")
                nc.vector.reciprocal_approx_fast(rz2[:], ps_rz[:, :, :NI])
                y_sb = pool_y.tile([128, 4, NI], bf, tag="y")
                nc.vector.tensor_mul(y_sb[:], ps_y[:, :, :NI], rz2[:])
                pending = (y_sb, g)

            emit_fin(*pending)

    nc.compile()
    return nc


# ------------------------------------------------------------- run helpers
_CACHE = {}
_LOCK = threading.Lock()
LAST_RESULT = None


def _get_nc(n_windows: int):
    with _LOCK:
        if n_windows not in _CACHE:
            _CACHE[n_windows] = _build_bass(n_windows)
        return _CACHE[n_windows]


def kernel(x, w_qkv, w_out, bias_table):
    from concourse.bass_utils import run_bass_kernel_spmd

    global LAST_RESULT
    x = np.asarray(x, dtype=np.float32)
    b, X, Y, Z, w1, w2, w3, d = x.shape
    B = b * X * Y * Z
    assert B == B_FULL and w1 * w2 * w3 == N_TOK and d == D
    w_core = B // N_CORES
    nt = w_core * N_TOK

    consts = _host_constants(np.asarray(w_qkv, np.float32),
                             np.asarray(w_out, np.float32),
                             np.asarray(bias_table, np.float32))
    nc = _get_nc(w_core)

    xf = np.ascontiguousarray(x.reshape(B * N_TOK, D))
    in_maps = []
    for c in range(N_CORES):
        m = {"xT": _host_xT(xf[c * nt: (c + 1) * nt], w_core),
             "wq": consts["wq"], "wk": consts["wk"], "wv": consts["wv"],
             "wo": consts["wo"], "eb4": consts["eb4"],
             "ones32": consts["ones32"]}
        in_maps.append(m)

    res = run_bass_kernel_spmd(nc, in_maps, core_ids=list(range(N_CORES)))
    LAST_RESULT = res
    out = np.concatenate([r["out"] for r in res.results], axis=0)
    return out.reshape(x.shape)


# revision 9
# speedup vs baseline: 1.1470x; 1.1470x over previous
"""Trainium2 Bass kernel v4 for windowed multi-head attention (2.5D swin).

Problem (hardcoded from spec nn_Attention25d_86775519248925):
  x:          (4, 16, 16, 8, 7, 7, 1, 128) f32  -> B=8192 windows, n=49 tokens, d=128
  w_qkv:      (128, 384) f32
  w_out:      (128, 128) f32
  bias_table: (169, 4) f32
  out:        same shape as x

Sharding: pure data parallel over the fused window-batch axis across 8 cores.

v4 design (per core: 128 groups of 8 windows = 4 window-pairs,
token slots fully COMPACT: 98 = 2x49 per pair, no padding):
  - x is reshaped + bf16-cast on the HOST into xT [d=128, group, pair, 98]
    (a pure reshape/transpose -- tokens are contiguous), so the kernel
    needs no transposes, no cast-DMAs, no padding, and input HBM traffic
    is 0.38x of the f32 token-major original.
  - q,k: two shared-weight matmuls (N=392) into ONE 2-bank psum tile;
    qT copy on scalar, kT copy on vector (concurrent).
  - sim^T per (head, pair): one matmul, lhsT = kT 32-row head slice
    (tile_position=(32h,0)), 98-j output.  Two heads share one 2-bank
    psum tile (per-head banks satisfy the row-group/bank rule).
  - softmax per HEAD (fine-grained pipeline): exp on scalar (psum->sbuf
    bf16), multiply by host-precomputed exp(bias) on GPSIMD (sbuf-only
    engine; masking of cross-window entries is exact multiply-by-zero;
    |sim| <= ~0.4 so exp never overflows), then immediately the Z matmul
    (lhsT=ones[98,32], col-masked to partitions 32h..32h+32 -- broadcast
    over dh for free) and the 4 attn@v matmuls (K=98, col-masked).
  - reciprocal_approx_fast for 1/Z; applied in the psum->sbuf y copy as a
    single vector multiply.
  - final projection of group g is deferred until after group g+1's sim
    matmuls; outb copy is split scalar/vector halves.

PSUM banks (8): sim 2x2, qk 1x2, {v, fin, rz, y} ring 2x1.

Hardware constraints honored (probed in earlier versions):
  - concurrent tile-position matmuls from different row-groups must write
    different PSUM banks (per-head sim banks).
  - no PSUM accumulation chains across row-groups (attn@v contracts K=98
    in one matmul; cross-window attn entries are exactly 0).
  - GPSIMD cannot access PSUM (it only gets the sbuf-only eb multiply).
"""

import os
import sys
import threading

import numpy as np

for _p in ("/opt/trn_rl_repo", "/root/.axon_site/_ro/trn_rl_repo"):
    if os.path.isdir(_p) and _p not in sys.path:
        sys.path.insert(0, _p)

# ---------------------------------------------------------------- constants
WS = 7
N_TOK = 49            # tokens per window
D = 128
H = 4
DH = 32
SCALE = DH ** -0.5
B_FULL = 4 * 16 * 16 * 8   # 8192 windows
N_CORES = 8
NI = 2 * N_TOK        # 98 compact token slots per pair


def _rel_pos_bias(bias_table: np.ndarray) -> np.ndarray:
    """bias[h, i, j] from the 169x4 table (numpy copy of reference logic)."""
    pos = np.arange(WS)
    gi, gj = np.meshgrid(pos, pos, indexing="ij")
    grid = np.stack([gi.reshape(-1), gj.reshape(-1)], axis=-1)
    rel = grid[:, None, :] - grid[None, :, :] + (WS - 1)
    idx = rel[..., 0] * (2 * WS - 1) + rel[..., 1]            # (49, 49)
    b = bias_table[idx]                                       # (49, 49, 4)
    return np.transpose(b, (2, 0, 1)).astype(np.float32)      # (h, i, j)


def _host_constants(w_qkv, w_out, bias_table):
    import ml_dtypes
    bf = ml_dtypes.bfloat16
    wq = np.ascontiguousarray((w_qkv[:, :D] * SCALE).astype(bf))
    wk = np.ascontiguousarray(w_qkv[:, D:2 * D].astype(bf))
    wv = np.ascontiguousarray(w_qkv[:, 2 * D:].astype(bf))
    wo = np.ascontiguousarray(w_out.astype(bf))

    bias = _rel_pos_bias(np.asarray(bias_table, dtype=np.float32))  # (h,i,j)
    # eb4[j, h, p, i] = exp(bias) on the window-diagonal, 0 elsewhere
    # (cross-window masking by multiply).  j, i in [0, 98), 49 per window.
    b4 = np.zeros((NI, H, 4, NI), dtype=np.float32)
    ebT = np.exp(np.transpose(bias, (0, 2, 1)))               # (h, j_tok, i_tok)
    for w in range(2):
        b4[N_TOK * w: N_TOK * (w + 1), :, :, N_TOK * w: N_TOK * (w + 1)] = \
            ebT.transpose(1, 0, 2)[:, :, None, :]
    eb4 = b4.astype(bf)
    ones32 = np.ones((NI, DH), dtype=bf)
    return dict(wq=wq, wk=wk, wv=wv, wo=wo, eb4=eb4, ones32=ones32)


def _host_xT(x_tokens: np.ndarray, n_windows: int) -> np.ndarray:
    """xT [128(d), n_groups, 4(pair), 98(tok)] bf16 from token-major
    x [nt, 128] f32 for one core -- a pure reshape/transpose/cast."""
    import ml_dtypes
    bf = ml_dtypes.bfloat16
    n_groups = n_windows // 8
    xT = np.ascontiguousarray(
        x_tokens.reshape(n_groups, 4, NI, D).transpose(3, 0, 1, 2).astype(bf))
    return xT


def _build_bass(n_windows: int):
    """Build the Bass/Tile program for one core processing n_windows windows."""
    import concourse.bacc as bacc
    import concourse.bass as bass
    import concourse.mybir as mybir
    import concourse.tile as tile

    f32 = mybir.dt.float32
    bf = mybir.dt.bfloat16
    NT = n_windows * N_TOK
    n_groups = n_windows // 8
    assert n_windows % 8 == 0

    nc = bacc.Bacc("TRN2", target_bir_lowering=False, debug=False,
                   enable_asserts=False)

    xT_d = nc.dram_tensor("xT", [D, n_groups, 4, NI], bf, kind="ExternalInput")
    out_t = nc.dram_tensor("out", [NT, D], f32, kind="ExternalOutput")
    wq_d = nc.dram_tensor("wq", [D, D], bf, kind="ExternalInput")
    wk_d = nc.dram_tensor("wk", [D, D], bf, kind="ExternalInput")
    wv_d = nc.dram_tensor("wv", [D, D], bf, kind="ExternalInput")
    wo_d = nc.dram_tensor("wo", [D, D], bf, kind="ExternalInput")
    eb_d = nc.dram_tensor("eb4", [NI, H, 4, NI], bf, kind="ExternalInput")
    ones_d = nc.dram_tensor("ones32", [NI, DH], bf, kind="ExternalInput")

    with tile.TileContext(nc) as tc:
        with (
            tc.tile_pool(name="singles", bufs=1) as singles,
            tc.tile_pool(name="xt", bufs=4) as pool_xt,
            tc.tile_pool(name="qk", bufs=3) as pool_qk,
            tc.tile_pool(name="vsb", bufs=3) as pool_v,
            tc.tile_pool(name="attn", bufs=8) as pool_attn,
            tc.tile_pool(name="rz", bufs=2) as pool_rz,
            tc.tile_pool(name="ysb", bufs=3) as pool_y,
            tc.tile_pool(name="outb", bufs=3) as pool_out,
            tc.tile_pool(name="psS", bufs=2, space="PSUM") as pool_sim,
            tc.tile_pool(name="psQK", bufs=1, space="PSUM") as pool_pqk,
            tc.tile_pool(name="psV", bufs=2, space="PSUM") as pool_vfy,
        ):
            wq_sb = singles.tile([D, D], bf, tag="wq")
            wk_sb = singles.tile([D, D], bf, tag="wk")
            wv_sb = singles.tile([D, D], bf, tag="wv")
            wo_sb = singles.tile([D, D], bf, tag="wo")
            eb_sb = singles.tile([NI, H, 4, NI], bf, tag="eb")
            ones_sb = singles.tile([NI, DH], bf, tag="ones")
            for sb, dr in ((wq_sb, wq_d), (wk_sb, wk_d), (wv_sb, wv_d),
                           (wo_sb, wo_d), (eb_sb, eb_d), (ones_sb, ones_d)):
                nc.sync.dma_start(out=sb[:], in_=dr[:])

            def emit_fin(y_sb, g):
                ps_f = pool_vfy.tile([128, 4, 128], f32, tag="v")
                for p in range(4):
                    nc.tensor.matmul(ps_f[:NI, p, :], y_sb[:, p, :], wo_sb[:])
                outb = pool_out.tile([NI, 4, D], f32, tag="outb")
                nc.scalar.copy(outb[:, 0:2, :], ps_f[:NI, 0:2, :])
                nc.vector.tensor_copy(outb[:, 2:4, :], ps_f[:NI, 2:4, :])
                tok0 = g * 392
                for p_ in range(2):
                    od_ap = bass.AP(
                        tensor=out_t, offset=(tok0 + p_ * N_TOK) * D,
                        ap=[[D, N_TOK], [2 * N_TOK * D, 4], [1, D]])
                    nc.sync.dma_start(
                        out=od_ap, in_=outb[N_TOK * p_: N_TOK * (p_ + 1)])

            def emit_zav(attn_tiles, v_sb, g):
                """Z + attn@v + 1/Z of group g (deferred to iteration g+1).
                PE: 4 Z + 16 av matmuls; vector: recip + ymul."""
                ps_rz = pool_vfy.tile([128, 4, 128], f32, tag="v")
                ps_y = pool_vfy.tile([128, 4, 128], f32, tag="v")
                for hh in range(2):
                    for hi in range(2):
                        h = 2 * hh + hi
                        nc.tensor.matmul(
                            ps_rz[DH * h: DH * (h + 1), :, :NI],
                            ones_sb[:], attn_tiles[hh][:, hi, :, :],
                            tile_position=(0, DH * h), start=True, stop=True)
                    for p in range(4):
                        for hi in range(2):
                            h = 2 * hh + hi
                            nc.tensor.matmul(
                                ps_y[DH * h: DH * (h + 1), p, :NI],
                                v_sb[:, p, DH * h: DH * (h + 1)],
                                attn_tiles[hh][:, hi, p, :],
                                tile_position=(0, DH * h), start=True, stop=True)
                rz2 = pool_rz.tile([128, 4, NI], f32, tag="rz")
                nc.vector.reciprocal_approx_fast(rz2[:], ps_rz[:, :, :NI])
                y_sb = pool_y.tile([128, 4, NI], bf, tag="y")
                nc.vector.tensor_mul(y_sb[:], ps_y[:, :, :NI], rz2[:])
                return (y_sb, g)

            zav_pend = None
            fin_pend = None
            for g in range(n_groups):
                # ---- input: straight DMA of host-pretransposed x ---------
                xT = pool_xt.tile([128, 4, NI], bf, tag="xt")
                nc.sync.dma_start(out=xT[:], in_=xT_d[:, g, :, :])

                # ---- q, k into one 2-bank tile; concurrent copies --------
                ps_qk = pool_pqk.tile([128, 2, 4, 128], f32, tag="qk")
                nc.tensor.matmul(ps_qk[:, 0, :, :NI], wq_sb[:], xT[:])
                nc.tensor.matmul(ps_qk[:, 1, :, :NI], wk_sb[:], xT[:])
                qT = pool_qk.tile([128, 4, NI], bf, tag="qT")
                nc.scalar.copy(qT[:], ps_qk[:, 0, :, :NI])
                kT = pool_qk.tile([128, 4, NI], bf, tag="kT")
                nc.vector.tensor_copy(kT[:], ps_qk[:, 1, :, :NI])

                # ---- Z + attn@v of the previous group fills the PE while
                # ---- this group's qT/kT copies run ------------------------
                y_prev = emit_zav(*zav_pend) if zav_pend is not None else None

                # ---- sim^T: heads interleaved for row-group concurrency --
                sim_tiles = []
                for hh in range(2):           # head pair (2hh, 2hh+1)
                    ps_sim = pool_sim.tile([128, 2, 4, 128], f32, tag="s")
                    for p in range(4):
                        for hi in range(2):
                            h = 2 * hh + hi
                            nc.tensor.matmul(
                                ps_sim[:NI, hi, p, :NI],
                                kT[DH * h: DH * (h + 1), p, :],
                                qT[DH * h: DH * (h + 1), p, :],
                                tile_position=(DH * h, 0), start=True, stop=True)
                    sim_tiles.append(ps_sim)

                # ---- v token-major (pair-stationary xT) ------------------
                ps_v = pool_vfy.tile([128, 4, 128], f32, tag="v")
                for p in range(4):
                    nc.tensor.matmul(ps_v[:NI, p, :], xT[:, p, :], wv_sb[:])
                v_sb = pool_v.tile([NI, 4, 128], bf, tag="v")
                nc.vector.tensor_copy(v_sb[:], ps_v[:NI, :, :])

                # ---- softmax numerator: exp + eb per head-pair -----------
                attn_tiles = []
                for hh in range(2):
                    attn = pool_attn.tile([NI, 2, 4, NI], bf, tag="attn")
                    nc.scalar.activation(attn[:], sim_tiles[hh][:NI, :, :, :NI],
                                         mybir.ActivationFunctionType.Exp)
                    nc.gpsimd.tensor_mul(
                        attn[:], attn[:], eb_sb[:, 2 * hh: 2 * hh + 2, :, :])
                    attn_tiles.append(attn)

                # ---- deferred final projection (two groups back) ---------
                if fin_pend is not None:
                    emit_fin(*fin_pend)
                fin_pend = y_prev
                zav_pend = (attn_tiles, v_sb, g)

            y_last = emit_zav(*zav_pend)
            if fin_pend is not None:
                emit_fin(*fin_pend)
            emit_fin(*y_last)

    nc.compile()
    return nc


# ------------------------------------------------------------- run helpers
_CACHE = {}
_LOCK = threading.Lock()
LAST_RESULT = None


def _get_nc(n_windows: int):
    with _LOCK:
        if n_windows not in _CACHE:
            _CACHE[n_windows] = _build_bass(n_windows)
        return _CACHE[n_windows]


def kernel(x, w_qkv, w_out, bias_table):
    from concourse.bass_utils import run_bass_kernel_spmd

    global LAST_RESULT
    x = np.asarray(x, dtype=np.float32)
    b, X, Y, Z, w1, w2, w3, d = x.shape
    B = b * X * Y * Z
    assert B == B_FULL and w1 * w2 * w3 == N_TOK and d == D
    w_core = B // N_CORES
    nt = w_core * N_TOK

    consts = _host_constants(np.asarray(w_qkv, np.float32),
                             np.asarray(w_out, np.float32),
                             np.asarray(bias_table, np.float32))
    nc = _get_nc(w_core)

    xf = np.ascontiguousarray(x.reshape(B * N_TOK, D))
    in_maps = []
    for c in range(N_CORES):
        m = {"xT": _host_xT(xf[c * nt: (c + 1) * nt], w_core),
             "wq": consts["wq"], "wk": consts["wk"], "wv": consts["wv"],
             "wo": consts["wo"], "eb4": consts["eb4"],
             "ones32": consts["ones32"]}
        in_maps.append(m)

    res = run_bass_kernel_spmd(nc, in_maps, core_ids=list(range(N_CORES)))
    LAST_RESULT = res
    out = np.concatenate([r["out"] for r in res.results], axis=0)
    return out.reshape(x.shape)
